# revision 3
# baseline (speedup 1.0000x reference)
"""GCNConv on 8 Trainium2 NeuronCores.

v4: DVE was the binding engine of the original pipeline. The PSUM->SBUF
aggregation copy and the bias add now run on the Activation engine (bias via
a K=128 ones x bias matmul accumulated under the weight matmul in PSUM),
leaving DVE with only the selection-matrix builds: 413us -> ~252us per pass.


out = in_norm * (A @ (out_norm * (x @ W))) + bias, A = unweighted CSR adjacency.

Sharding: each core owns 1/8 of the destination rows and receives the FULL x
in its own HBM (host-side input staging), so no collectives are needed. Per
128-dest window a core gathers the window's source rows from x via int16
dma_gather (4 residue-bucketed calls over a 2048B-strided table view to dodge
the int16 index range), aggregates them with selection-matrix matmuls on the
PE (PSUM-accumulated per window), then applies the 128x128 weight and bias.
Degree norms are folded into per-edge weights (constant 1/16 into W when
degrees are uniform). Bucket sizes differ per core, so each core gets its own
NEFF; the 8 single-device executables run concurrently via PJRT.
"""
import math
import numpy as np

import jax

import concourse.bass as bass
import concourse.bacc as bacc
import concourse.mybir as mybir
from concourse.tile import TileContext
from concourse.bass2jax import (
    _bass_exec_p, install_neuronx_cc_hook, partition_id_tensor,
)

N_CORES = 8
C = 128
P = 128
f32 = mybir.dt.float32
i16 = mybir.dt.int16

import os as _os
_SENTINEL = 300.0  # destid sentinel -> no is_equal match -> zero S row
FP16 = _os.environ.get("GCN_FP16", "0") == "1"  # fp16 path is slow on this HW
GP_BUFS = int(_os.environ.get("GCN_GP_BUFS", "3"))  # 3 = measured optimum
TAB16 = _os.environ.get("GCN_TAB16", "0") == "1"  # fp16 gather table
ACT = _os.environ.get("GCN_ACT", "1") == "1"      # PSUM copies+bias off DVE
SB = _os.environ.get("GCN_SBUILD", "bcast")       # "bcast" | "packed"
NOSB = _os.environ.get("GCN_NOSB", "0") == "1"    # const S probe (wrong output)
CONVDVE = _os.environ.get("GCN_CONVDVE", "0") == "1"  # g16 convert on DVE
DID_NP_F16 = True  # did/iota/v staged fp16 when FP16

f16 = (mybir.dt.bfloat16 if _os.environ.get("GCN_BF16", "0") == "1"
       else mybir.dt.float16)

_CACHE = {}


def _np_half():
    if _os.environ.get("GCN_BF16", "0") == "1":
        import ml_dtypes
        return ml_dtypes.bfloat16
    return np.float16


def _wrap_idx(idx):
    """[n] int -> [128, n/16] int16 wrapped + replicated for dma_gather."""
    w = np.asarray(idx, np.int16).reshape(-1, 16).T
    return np.ascontiguousarray(np.tile(w, (8, 1)))


def _prep_core(c, n_dest, rowptr, colind, v_edge):
    """Host-side metadata for core c: per (window, residue) idx + destid."""
    d0 = c * n_dest
    windows = []
    for w in range(math.ceil(n_dest / P)):
        wd0 = d0 + w * P
        wd1 = min(wd0 + P, d0 + n_dest)
        e0, e1 = int(rowptr[wd0]), int(rowptr[wd1])
        srcs = colind[e0:e1].astype(np.int64)
        dloc = np.searchsorted(rowptr[wd0:wd1 + 1] - rowptr[wd0],
                               np.arange(e1 - e0), side="right") - 1
        vv = v_edge[e0:e1] if v_edge is not None else None
        res = srcs & 3
        q = srcs >> 2
        calls = []
        for r in range(4):
            m = res == r
            dr, qr = dloc[m], q[m]
            order = np.argsort(dr, kind="stable")
            dr, qr = dr[order], qr[order]
            vr = vv[m][order] if vv is not None else None
            b = len(qr)
            n_pad = max(P, ((b + P - 1) // P) * P)
            qp = np.full(n_pad, -1, np.int64)
            qp[:b] = qr
            dp = np.full(n_pad, _SENTINEL, np.float32)
            dp[:b] = dr
            vp = None
            if vr is not None:
                vp = np.zeros(n_pad, np.float32)
                vp[:b] = vr
            calls.append((qp, dp, vp, b))
        windows.append(calls)
    return windows


def _build_core(n_dest, n_table_rows, windows, uniform, nq=4, repeat=1):
    """Build one core's Bacc kernel."""
    nwin = len(windows)
    idx_parts, did_parts, v_parts = [], [], []
    slices = []  # per (w, r): (idx_col0, n, reg, did_col0, ngr)
    cum_slots = 0
    for calls in windows:
        for (qp, dp, vp, b) in calls:
            n = len(qp)
            slices.append((cum_slots // 16, n, b, cum_slots // 128, n // P))
            cum_slots += n
            idx_parts.append(qp)
            did_parts.append(dp)
            if vp is not None:
                v_parts.append(vp)
    idx_all = np.concatenate(idx_parts)
    did_all = np.concatenate(did_parts)
    tot_cols = len(idx_all) // 16
    dcols = len(did_all) // 128
    ngr_max = max(s[4] for s in slices)

    nc = bacc.Bacc("TRN2", target_bir_lowering=False, num_devices=1,
                   num_swdge_queues=nq)
    x = nc.dram_tensor("x", [n_table_rows, 4 * C], f16 if TAB16 else f32, kind="ExternalInput")
    idxd = nc.dram_tensor("idx", [128, tot_cols], i16, kind="ExternalInput")
    sdt0 = f16 if FP16 else f32
    didd = nc.dram_tensor("did", [128, dcols], sdt0, kind="ExternalInput")
    wtd = nc.dram_tensor("wt", [C, C], f32, kind="ExternalInput")
    biasd = nc.dram_tensor("biasb", [128, C], f32, kind="ExternalInput")
    vd = None
    if not uniform:
        vd = nc.dram_tensor("v", [128, dcols], sdt0, kind="ExternalInput")
    outd = nc.dram_tensor("out", [n_dest, C], f32, kind="ExternalOutput")

    with TileContext(nc) as tc:
        with tc.tile_pool(name="const", bufs=1) as cp, \
             tc.tile_pool(name="gp", bufs=GP_BUFS) as gp, \
             tc.tile_pool(name="g16p", bufs=2) as g16p, \
             tc.tile_pool(name="sp", bufs=3) as spool, \
             tc.tile_pool(name="op", bufs=2) as op, \
             tc.tile_pool(name="ps", bufs=4, space="PSUM") as ps, \
             tc.tile_pool(name="ps2", bufs=2, space="PSUM") as ps2:
            idx_t = cp.tile([128, tot_cols], i16, name="idxt")
            nc.sync.dma_start(idx_t[:], idxd[:])
            did_t = cp.tile([128, dcols], sdt0, name="didt")
            nc.sync.dma_start(did_t[:], didd[:])
            wt_t = cp.tile([C, C], f32, name="wtt")
            nc.sync.dma_start(wt_t[:], wtd[:])
            wt16_t = None
            if FP16:
                wt16_t = cp.tile([C, C], f16, name="wt16t")
                nc.scalar.copy(out=wt16_t[:], in_=wt_t[:])
            bias_t = cp.tile([128, C], f32, name="biast")
            nc.sync.dma_start(bias_t[:], biasd[:])
            if SB == "packed":
                iota_t = cp.tile([128, 128, ngr_max], sdt0, name="iotat")
                nc.gpsimd.iota(iota_t[:], pattern=[[1, 128], [0, ngr_max]],
                               base=0, channel_multiplier=0,
                               allow_small_or_imprecise_dtypes=True)
            else:
                iota_t = cp.tile([128, 128], sdt0, name="iotat")
                nc.gpsimd.iota(iota_t[:], pattern=[[1, 128]], base=0,
                               channel_multiplier=0,
                               allow_small_or_imprecise_dtypes=True)
            s_const = None
            if NOSB:
                s_const = cp.tile([128, ngr_max, 128], f16 if FP16 else f32,
                                  name="sconst")
                nc.vector.memset(s_const[:], 0.01)
            bias16_t = None
            ones_t = None
            if ACT:
                bias16_t = cp.tile([128, C], f16, name="bias16t")
                nc.scalar.copy(out=bias16_t[:], in_=bias_t[:])
                ones_t = cp.tile([128, C], f16, name="onest")
                nc.vector.memset(ones_t[:], 1.0 / 128.0)
            v_t = None
            if vd is not None:
                v_t = cp.tile([128, dcols], sdt0, name="vt")
                nc.sync.dma_start(v_t[:], vd[:])

            for rep in range(repeat):
              si = 0
              for w in range(nwin):
                  psum = ps.tile([128, 128], f32, name=f"ps{rep}_{w}", tag="psw",
                                 space="PSUM")
                  first = True
                  for r in range(4):
                      col0, n, reg, dcol0, ngr = slices[si]
                      si += 1
                      g = gp.tile([128, ngr_max, C], f16 if TAB16 else f32,
                                  name=f"g{rep}_{w}_{r}", tag=f"g{r}")
                      if rep == 0 and w < GP_BUFS:
                          # first use of each rotating buffer: clear SBUF
                          # garbage so padded slots contribute 0 (not 0*NaN)
                          nc.vector.memset(g[:], 0.0)
                      nc.gpsimd.dma_gather(
                          g[:, :ngr, :], x[:, r * C:(r + 1) * C],
                          idx_t[:, col0:col0 + n // 16],
                          n, reg, C, elem_step=4 * C,
                          queue_num=(w * 4 + r) % nq,
                      )
                      if FP16 and not TAB16:
                          g16 = g16p.tile([128, ngr_max, C], f16,
                                          name=f"h{rep}_{w}_{r}", tag=f"h{r}")
                          if CONVDVE:
                              nc.vector.tensor_copy(out=g16[:, :ngr, :],
                                                    in_=g[:, :ngr, :])
                          else:
                              nc.scalar.copy(out=g16[:, :ngr, :],
                                             in_=g[:, :ngr, :])
                          mm_lhs = g16
                      else:
                          mm_lhs = g
                      sdt = f16 if FP16 else f32
                      dslice = did_t[:, dcol0:dcol0 + ngr]
                      if NOSB:
                          s_t = s_const
                      elif SB == "packed":
                          s_t = spool.tile([128, 128, ngr_max], sdt,
                                           name=f"s{rep}_{w}_{r}", tag=f"s{r}")
                          din = bass.AP(dslice.tensor, dslice.offset,
                                        [dslice.ap[0], (0, 128),
                                         dslice.ap[1]])
                          nc.vector.tensor_tensor(
                              out=s_t[:, :, :ngr], in0=din,
                              in1=iota_t[:, :, :ngr],
                              op=mybir.AluOpType.is_equal)
                          if v_t is not None:
                              vslice = v_t[:, dcol0:dcol0 + ngr]
                              vin = bass.AP(vslice.tensor, vslice.offset,
                                            [vslice.ap[0], (0, 128),
                                             vslice.ap[1]])
                              nc.vector.tensor_tensor(
                                  out=s_t[:, :, :ngr], in0=s_t[:, :, :ngr],
                                  in1=vin, op=mybir.AluOpType.mult)
                      else:
                          s_t = spool.tile([128, ngr_max, 128], sdt,
                                           name=f"s{rep}_{w}_{r}", tag=f"s{r}")
                          din = bass.AP(dslice.tensor, dslice.offset,
                                        dslice.ap + [(0, 128)])
                          isl = iota_t[:, :]
                          iin = bass.AP(isl.tensor, isl.offset,
                                        [isl.ap[0], (0, ngr), isl.ap[1]])
                          nc.vector.tensor_tensor(
                              out=s_t[:, :ngr, :], in0=din, in1=iin,
                              op=mybir.AluOpType.is_equal)
                          if v_t is not None:
                              vslice = v_t[:, dcol0:dcol0 + ngr]
                              vin = bass.AP(vslice.tensor, vslice.offset,
                                            vslice.ap + [(0, 128)])
                              nc.vector.tensor_tensor(
                                  out=s_t[:, :ngr, :], in0=s_t[:, :ngr, :],
                                  in1=vin, op=mybir.AluOpType.mult)
                      for gi in range(ngr):
                          last = (r == 3) and (gi == ngr - 1)
                          rhs = (s_t[:, :, gi]
                                 if SB == "packed" and not NOSB
                                 else s_t[:, gi, :])
                          nc.tensor.matmul(
                              out=psum[:],
                              lhsT=mm_lhs[:, gi, :],
                              rhs=rhs,
                              start=first, stop=last)
                          first = False
                  adt = f16 if FP16 else f32
                  aggrT = op.tile([128, 128], adt, name=f"aggrT{rep}_{w}", tag="aggrT")
                  if ACT:
                      nc.scalar.copy(out=aggrT[:], in_=psum[:])
                  else:
                      nc.vector.tensor_copy(out=aggrT[:], in_=psum[:])
                  psum2 = ps2.tile([128, 128], f32, name=f"q{rep}_{w}", tag="psq",
                                   space="PSUM")
                  osb = op.tile([128, 128], f32, name=f"osb{rep}_{w}", tag="osb")
                  if ACT:
                      nc.tensor.matmul(out=psum2[:], lhsT=ones_t[:],
                                       rhs=bias16_t[:], start=True, stop=False)
                      nc.tensor.matmul(out=psum2[:], lhsT=aggrT[:],
                                       rhs=wt16_t[:] if FP16 else wt_t[:],
                                       start=False, stop=True)
                      nc.scalar.copy(out=osb[:], in_=psum2[:])
                  else:
                      nc.tensor.matmul(out=psum2[:], lhsT=aggrT[:],
                                       rhs=wt16_t[:] if FP16 else wt_t[:],
                                       start=True, stop=True)
                      nc.vector.tensor_tensor(out=osb[:], in0=psum2[:],
                                              in1=bias_t[:],
                                              op=mybir.AluOpType.add)
                  d0, d1 = w * P, min((w + 1) * P, n_dest)
                  nc.sync.dma_start(outd[d0:d1, :], osb[:d1 - d0, :])
    nc.compile()
    v_all = np.concatenate(v_parts) if v_parts else None
    return nc, idx_all, did_all, v_all


def _make_single_runner(nc):
    install_neuronx_cc_hook()
    pname = nc.partition_id_tensor.name if nc.partition_id_tensor else None
    in_names, out_names, out_avals, zero_outs = [], [], [], []
    for alloc in nc.m.functions[0].allocations:
        if not isinstance(alloc, mybir.MemoryLocationSet):
            continue
        name = alloc.memorylocations[0].name
        if alloc.kind == "ExternalInput":
            if name != pname:
                in_names.append(name)
        elif alloc.kind == "ExternalOutput":
            shape = tuple(alloc.tensor_shape)
            dtype = mybir.dt.np(alloc.dtype)
            out_avals.append(jax.core.ShapedArray(shape, dtype))
            zero_outs.append(np.zeros(shape, dtype))
            out_names.append(name)
    all_in = list(in_names) + list(out_names)
    if pname is not None:
        all_in.append(pname)

    def _body(*args):
        operands = list(args)
        if pname is not None:
            operands.append(partition_id_tensor())
        return tuple(_bass_exec_p.bind(
            *operands, out_avals=tuple(out_avals), in_names=tuple(all_in),
            out_names=tuple(out_names),
            lowering_input_output_aliases=(),
            sim_require_finite=True, sim_require_nnan=True, nc=nc))

    fn = jax.jit(_body, keep_unused=True)
    return fn, in_names, out_names, zero_outs


def _build_all(x, weight, bias, rowptr, colind, colptr):
    n_nodes = rowptr.shape[0] - 1
    n_dest = n_nodes // N_CORES

    deg_in = np.diff(rowptr).astype(np.float64)
    deg_out = np.diff(colptr).astype(np.float64)
    with np.errstate(divide="ignore"):
        in_norm = 1.0 / np.sqrt(deg_in)
        out_norm = 1.0 / np.sqrt(deg_out)
    n_used = min(colind.shape[0], int(rowptr[-1]))
    uniform = bool(np.all(deg_in == deg_in[0]) and np.all(deg_out == deg_out[0])
                   and np.isfinite(in_norm[0]) and np.isfinite(out_norm[0]))
    if uniform:
        v_edge = None
        w_eff = (weight * np.float32(in_norm[0] * out_norm[0])).astype(np.float32)
    else:
        row_of_edge = (np.searchsorted(rowptr, np.arange(n_used),
                                       side="right") - 1)
        v_edge = (out_norm[colind[:n_used]] * in_norm[row_of_edge]).astype(
            np.float32)
        w_eff = weight.astype(np.float32)

    n_pad_rows = ((n_nodes + 3) // 4) * 4
    n_table_rows = n_pad_rows // 4
    assert n_table_rows <= 32768, "int16 gather reach exceeded"

    bias_b = np.ascontiguousarray(np.tile(bias[None, :], (128, 1)).astype(np.float32))
    iota = np.ascontiguousarray(
        np.tile(np.arange(128, dtype=np.float32)[None, :], (128, 1)))

    cores = []
    for c in range(N_CORES):
        windows = _prep_core(c, n_dest, rowptr, colind[:n_used], v_edge)
        nc, idx_all, did_all, v_all = _build_core(
            n_dest, n_table_rows, windows, uniform)
        fn, in_names, out_names, zero_outs = _make_single_runner(nc)
        ddt = _np_half() if FP16 else np.float32
        in_map = {
            "idx": _wrap_idx(idx_all),
            "did": np.ascontiguousarray(did_all.reshape(-1, 128).T.astype(ddt)),
            "wt": w_eff,
            "biasb": bias_b,
        }
        if v_all is not None:
            in_map["v"] = np.ascontiguousarray(
                v_all.reshape(-1, 128).T.astype(ddt))
        cores.append((fn, in_names, out_names, zero_outs, in_map))
    return cores, n_pad_rows, deg_in


def get_runners(x, weight, bias, rowptr, colind, colptr):
    key = (x.shape, hash(rowptr.tobytes()), hash(colind.tobytes()),
           hash(colptr.tobytes()))
    if key not in _CACHE:
        _CACHE[key] = _build_all(x, weight, bias, rowptr, colind, colptr)
    return _CACHE[key]


def run_on_device(cores, x_view, bias, deg_in):
    futs = []
    for c, (fn, in_names, out_names, zero_outs, in_map) in enumerate(cores):
        dev = jax.devices()[c]
        full = dict(in_map, x=x_view)
        dev_in = [jax.device_put(np.asarray(full[n]), dev) for n in in_names]
        dev_zero = [jax.device_put(z, dev) for z in zero_outs]
        futs.append((fn(*dev_in, *dev_zero), out_names))
    results = []
    for (out_arrs, out_names) in futs:
        jax.block_until_ready(out_arrs)
        results.append(np.asarray(out_arrs[out_names.index("out")]))
    out = np.concatenate(results, axis=0)
    zero_deg = deg_in == 0
    if zero_deg.any():
        out[zero_deg] = (np.float32(0) * np.float32(np.inf)) + bias[None, :]
    return out


def kernel(x, weight, bias, rowptr, colind, colptr, rowind):
    x = np.ascontiguousarray(np.asarray(x, np.float32))
    weight = np.asarray(weight, np.float32)
    bias = np.asarray(bias, np.float32)
    rowptr = np.asarray(rowptr, np.int64)
    colind = np.asarray(colind, np.int64)
    colptr = np.asarray(colptr, np.int64)

    n_nodes = rowptr.shape[0] - 1
    cores, n_pad_rows, deg_in = get_runners(x, weight, bias, rowptr, colind,
                                            colptr)
    dt = _np_half() if TAB16 else np.float32
    if n_pad_rows == n_nodes:
        x_view = np.ascontiguousarray(x.astype(dt)).reshape(n_nodes // 4, 4 * C)
    else:
        xp = np.zeros((n_pad_rows, C), dt)
        xp[:n_nodes] = x
        x_view = xp.reshape(n_pad_rows // 4, 4 * C)
    return run_on_device(cores, x_view, bias, deg_in)



# revision 4
# speedup vs baseline: 1.4798x; 1.4798x over previous
"""GCNConv on 8 Trainium2 NeuronCores.

v4: DVE was the binding engine of the original pipeline. The PSUM->SBUF
aggregation copy and the bias add now run on the Activation engine (bias via
a K=128 ones x bias matmul accumulated under the weight matmul in PSUM),
leaving DVE with only the selection-matrix builds: 413us -> ~252us per pass.


out = in_norm * (A @ (out_norm * (x @ W))) + bias, A = unweighted CSR adjacency.

Sharding: each core owns 1/8 of the destination rows and receives the FULL x
in its own HBM (host-side input staging), so no collectives are needed. Per
128-dest window a core gathers the window's source rows from x via int16
dma_gather (4 residue-bucketed calls over a 2048B-strided table view to dodge
the int16 index range), aggregates them with selection-matrix matmuls on the
PE (PSUM-accumulated per window), then applies the 128x128 weight and bias.
Degree norms are folded into per-edge weights (constant 1/16 into W when
degrees are uniform). Bucket sizes differ per core, so each core gets its own
NEFF; the 8 single-device executables run concurrently via PJRT.
"""
import math
import numpy as np

import jax

import concourse.bass as bass
import concourse.bacc as bacc
import concourse.mybir as mybir
from concourse.tile import TileContext
from concourse.bass2jax import (
    _bass_exec_p, install_neuronx_cc_hook, partition_id_tensor,
)

N_CORES = 8
C = 128
P = 128
f32 = mybir.dt.float32
i16 = mybir.dt.int16

import os as _os
_SENTINEL = 300.0  # destid sentinel -> no is_equal match -> zero S row
FP16 = _os.environ.get("GCN_FP16", "0") == "1"  # fp16 path is slow on this HW
GP_BUFS = int(_os.environ.get("GCN_GP_BUFS", "3"))  # 3 = measured optimum
TAB16 = _os.environ.get("GCN_TAB16", "0") == "1"  # fp16 gather table
ACT = _os.environ.get("GCN_ACT", "1") == "1"      # PSUM copies+bias off DVE
SB = _os.environ.get("GCN_SBUILD", "bcast")       # "bcast" | "packed"
NOSB = _os.environ.get("GCN_NOSB", "0") == "1"    # const S probe (wrong output)
CONVDVE = _os.environ.get("GCN_CONVDVE", "0") == "1"  # g16 convert on DVE
DID_NP_F16 = True  # did/iota/v staged fp16 when FP16

f16 = (mybir.dt.bfloat16 if _os.environ.get("GCN_BF16", "0") == "1"
       else mybir.dt.float16)

_CACHE = {}


def _np_half():
    if _os.environ.get("GCN_BF16", "0") == "1":
        import ml_dtypes
        return ml_dtypes.bfloat16
    return np.float16


def _wrap_idx(idx):
    """[n] int -> [128, n/16] int16 wrapped + replicated for dma_gather."""
    w = np.asarray(idx, np.int16).reshape(-1, 16).T
    return np.ascontiguousarray(np.tile(w, (8, 1)))


def _prep_core(c, n_dest, rowptr, colind, v_edge):
    """Host-side metadata for core c: per (window, residue) idx + destid."""
    d0 = c * n_dest
    windows = []
    for w in range(math.ceil(n_dest / P)):
        wd0 = d0 + w * P
        wd1 = min(wd0 + P, d0 + n_dest)
        e0, e1 = int(rowptr[wd0]), int(rowptr[wd1])
        srcs = colind[e0:e1].astype(np.int64)
        dloc = np.searchsorted(rowptr[wd0:wd1 + 1] - rowptr[wd0],
                               np.arange(e1 - e0), side="right") - 1
        vv = v_edge[e0:e1] if v_edge is not None else None
        res = srcs & 3
        q = srcs >> 2
        calls = []
        for r in range(4):
            m = res == r
            dr, qr = dloc[m], q[m]
            order = np.argsort(dr, kind="stable")
            dr, qr = dr[order], qr[order]
            vr = vv[m][order] if vv is not None else None
            b = len(qr)
            n_pad = max(P, ((b + P - 1) // P) * P)
            qp = np.full(n_pad, -1, np.int64)
            qp[:b] = qr
            dp = np.full(n_pad, _SENTINEL, np.float32)
            dp[:b] = dr
            vp = None
            if vr is not None:
                vp = np.zeros(n_pad, np.float32)
                vp[:b] = vr
            calls.append((qp, dp, vp, b))
        windows.append(calls)
    return windows


def _build_core(n_dest, n_table_rows, windows, uniform, nq=4, repeat=1):
    """Build one core's Bacc kernel."""
    nwin = len(windows)
    idx_parts, did_parts, v_parts = [], [], []
    slices = []  # per (w, r): (idx_col0, n, reg, did_col0, ngr)
    cum_slots = 0
    for calls in windows:
        for (qp, dp, vp, b) in calls:
            n = len(qp)
            slices.append((cum_slots // 16, n, b, cum_slots // 128, n // P))
            cum_slots += n
            idx_parts.append(qp)
            did_parts.append(dp)
            if vp is not None:
                v_parts.append(vp)
    idx_all = np.concatenate(idx_parts)
    did_all = np.concatenate(did_parts)
    tot_cols = len(idx_all) // 16
    dcols = len(did_all) // 128
    ngr_max = max(s[4] for s in slices)

    nc = bacc.Bacc("TRN2", target_bir_lowering=False, num_devices=1,
                   num_swdge_queues=nq)
    x = nc.dram_tensor("x", [n_table_rows, 4 * C], f16 if TAB16 else f32, kind="ExternalInput")
    idxd = nc.dram_tensor("idx", [128, tot_cols], i16, kind="ExternalInput")
    sdt0 = f16 if FP16 else f32
    didd = nc.dram_tensor("did", [128, dcols], sdt0, kind="ExternalInput")
    wtd = nc.dram_tensor("wt", [C, C], f32, kind="ExternalInput")
    biasd = nc.dram_tensor("biasb", [128, C], f32, kind="ExternalInput")
    vd = None
    if not uniform:
        vd = nc.dram_tensor("v", [128, dcols], sdt0, kind="ExternalInput")
    outd = nc.dram_tensor("out", [n_dest, C], f32, kind="ExternalOutput")

    with TileContext(nc) as tc:
        with tc.tile_pool(name="const", bufs=1) as cp, \
             tc.tile_pool(name="gp", bufs=GP_BUFS) as gp, \
             tc.tile_pool(name="g16p", bufs=2) as g16p, \
             tc.tile_pool(name="sp", bufs=3) as spool, \
             tc.tile_pool(name="op", bufs=2) as op, \
             tc.tile_pool(name="ps", bufs=6, space="PSUM") as ps, \
             tc.tile_pool(name="ps2", bufs=2, space="PSUM") as ps2:
            idx_t = cp.tile([128, tot_cols], i16, name="idxt")
            nc.sync.dma_start(idx_t[:], idxd[:])
            did_t = cp.tile([128, dcols], sdt0, name="didt")
            nc.sync.dma_start(did_t[:], didd[:])
            wt_t = cp.tile([C, C], f32, name="wtt")
            nc.sync.dma_start(wt_t[:], wtd[:])
            wt16_t = None
            if FP16:
                wt16_t = cp.tile([C, C], f16, name="wt16t")
                nc.scalar.copy(out=wt16_t[:], in_=wt_t[:])
            bias_t = cp.tile([128, C], f32, name="biast")
            nc.sync.dma_start(bias_t[:], biasd[:])
            if SB == "packed":
                iota_t = cp.tile([128, 128, ngr_max], sdt0, name="iotat")
                nc.gpsimd.iota(iota_t[:], pattern=[[1, 128], [0, ngr_max]],
                               base=0, channel_multiplier=0,
                               allow_small_or_imprecise_dtypes=True)
            else:
                iota_t = cp.tile([128, 128], sdt0, name="iotat")
                nc.gpsimd.iota(iota_t[:], pattern=[[1, 128]], base=0,
                               channel_multiplier=0,
                               allow_small_or_imprecise_dtypes=True)
            s_const = None
            if NOSB:
                s_const = cp.tile([128, ngr_max, 128], f16 if FP16 else f32,
                                  name="sconst")
                nc.vector.memset(s_const[:], 0.01)
            bias16_t = None
            ones_t = None
            if ACT:
                bias16_t = cp.tile([128, C], f16, name="bias16t")
                nc.scalar.copy(out=bias16_t[:], in_=bias_t[:])
                ones_t = cp.tile([128, C], f16, name="onest")
                nc.vector.memset(ones_t[:], 1.0 / 128.0)
            v_t = None
            if vd is not None:
                v_t = cp.tile([128, dcols], sdt0, name="vt")
                nc.sync.dma_start(v_t[:], vd[:])

            for rep in range(repeat):
              si = 0
              for w in range(nwin):
                  psum = ps.tile([128, 128], f32, name=f"ps{rep}_{w}", tag="psw",
                                 space="PSUM")
                  first = True
                  for r in range(4):
                      col0, n, reg, dcol0, ngr = slices[si]
                      si += 1
                      g = gp.tile([128, ngr_max, C], f16 if TAB16 else f32,
                                  name=f"g{rep}_{w}_{r}", tag=f"g{r}")
                      if rep == 0 and w < GP_BUFS:
                          # first use of each rotating buffer: clear SBUF
                          # garbage so padded slots contribute 0 (not 0*NaN)
                          nc.vector.memset(g[:], 0.0)
                      nc.gpsimd.dma_gather(
                          g[:, :ngr, :], x[:, r * C:(r + 1) * C],
                          idx_t[:, col0:col0 + n // 16],
                          n, reg, C, elem_step=4 * C,
                          queue_num=(w * 4 + r) % nq,
                      )
                      if FP16 and not TAB16:
                          g16 = g16p.tile([128, ngr_max, C], f16,
                                          name=f"h{rep}_{w}_{r}", tag=f"h{r}")
                          if CONVDVE:
                              nc.vector.tensor_copy(out=g16[:, :ngr, :],
                                                    in_=g[:, :ngr, :])
                          else:
                              nc.scalar.copy(out=g16[:, :ngr, :],
                                             in_=g[:, :ngr, :])
                          mm_lhs = g16
                      else:
                          mm_lhs = g
                      sdt = f16 if FP16 else f32
                      dslice = did_t[:, dcol0:dcol0 + ngr]
                      if NOSB:
                          s_t = s_const
                      elif SB == "packed":
                          s_t = spool.tile([128, 128, ngr_max], sdt,
                                           name=f"s{rep}_{w}_{r}", tag=f"s{r}")
                          din = bass.AP(dslice.tensor, dslice.offset,
                                        [dslice.ap[0], (0, 128),
                                         dslice.ap[1]])
                          nc.vector.tensor_tensor(
                              out=s_t[:, :, :ngr], in0=din,
                              in1=iota_t[:, :, :ngr],
                              op=mybir.AluOpType.is_equal)
                          if v_t is not None:
                              vslice = v_t[:, dcol0:dcol0 + ngr]
                              vin = bass.AP(vslice.tensor, vslice.offset,
                                            [vslice.ap[0], (0, 128),
                                             vslice.ap[1]])
                              nc.vector.tensor_tensor(
                                  out=s_t[:, :, :ngr], in0=s_t[:, :, :ngr],
                                  in1=vin, op=mybir.AluOpType.mult)
                      else:
                          s_t = spool.tile([128, ngr_max, 128], sdt,
                                           name=f"s{rep}_{w}_{r}", tag=f"s{r}")
                          din = bass.AP(dslice.tensor, dslice.offset,
                                        dslice.ap + [(0, 128)])
                          isl = iota_t[:, :]
                          iin = bass.AP(isl.tensor, isl.offset,
                                        [isl.ap[0], (0, ngr), isl.ap[1]])
                          nc.vector.tensor_tensor(
                              out=s_t[:, :ngr, :], in0=din, in1=iin,
                              op=mybir.AluOpType.is_equal)
                          if v_t is not None:
                              vslice = v_t[:, dcol0:dcol0 + ngr]
                              vin = bass.AP(vslice.tensor, vslice.offset,
                                            vslice.ap + [(0, 128)])
                              nc.vector.tensor_tensor(
                                  out=s_t[:, :ngr, :], in0=s_t[:, :ngr, :],
                                  in1=vin, op=mybir.AluOpType.mult)
                      for gi in range(ngr):
                          last = (r == 3) and (gi == ngr - 1)
                          rhs = (s_t[:, :, gi]
                                 if SB == "packed" and not NOSB
                                 else s_t[:, gi, :])
                          nc.tensor.matmul(
                              out=psum[:],
                              lhsT=mm_lhs[:, gi, :],
                              rhs=rhs,
                              start=first, stop=last)
                          first = False
                  adt = f16 if FP16 else f32
                  aggrT = op.tile([128, 128], adt, name=f"aggrT{rep}_{w}", tag="aggrT")
                  if ACT:
                      nc.scalar.copy(out=aggrT[:], in_=psum[:])
                  else:
                      nc.vector.tensor_copy(out=aggrT[:], in_=psum[:])
                  psum2 = ps2.tile([128, 128], f32, name=f"q{rep}_{w}", tag="psq",
                                   space="PSUM")
                  osb = op.tile([128, 128], f32, name=f"osb{rep}_{w}", tag="osb")
                  if ACT:
                      nc.tensor.matmul(out=psum2[:], lhsT=ones_t[:],
                                       rhs=bias16_t[:], start=True, stop=False)
                      nc.tensor.matmul(out=psum2[:], lhsT=aggrT[:],
                                       rhs=wt16_t[:] if FP16 else wt_t[:],
                                       start=False, stop=True)
                      nc.scalar.copy(out=osb[:], in_=psum2[:])
                  else:
                      nc.tensor.matmul(out=psum2[:], lhsT=aggrT[:],
                                       rhs=wt16_t[:] if FP16 else wt_t[:],
                                       start=True, stop=True)
                      nc.vector.tensor_tensor(out=osb[:], in0=psum2[:],
                                              in1=bias_t[:],
                                              op=mybir.AluOpType.add)
                  d0, d1 = w * P, min((w + 1) * P, n_dest)
                  nc.sync.dma_start(outd[d0:d1, :], osb[:d1 - d0, :])
    nc.compile()
    v_all = np.concatenate(v_parts) if v_parts else None
    return nc, idx_all, did_all, v_all


def _make_single_runner(nc):
    install_neuronx_cc_hook()
    pname = nc.partition_id_tensor.name if nc.partition_id_tensor else None
    in_names, out_names, out_avals, zero_outs = [], [], [], []
    for alloc in nc.m.functions[0].allocations:
        if not isinstance(alloc, mybir.MemoryLocationSet):
            continue
        name = alloc.memorylocations[0].name
        if alloc.kind == "ExternalInput":
            if name != pname:
                in_names.append(name)
        elif alloc.kind == "ExternalOutput":
            shape = tuple(alloc.tensor_shape)
            dtype = mybir.dt.np(alloc.dtype)
            out_avals.append(jax.core.ShapedArray(shape, dtype))
            zero_outs.append(np.zeros(shape, dtype))
            out_names.append(name)
    all_in = list(in_names) + list(out_names)
    if pname is not None:
        all_in.append(pname)

    def _body(*args):
        operands = list(args)
        if pname is not None:
            operands.append(partition_id_tensor())
        return tuple(_bass_exec_p.bind(
            *operands, out_avals=tuple(out_avals), in_names=tuple(all_in),
            out_names=tuple(out_names),
            lowering_input_output_aliases=(),
            sim_require_finite=True, sim_require_nnan=True, nc=nc))

    fn = jax.jit(_body, keep_unused=True)
    return fn, in_names, out_names, zero_outs


def _build_all(x, weight, bias, rowptr, colind, colptr):
    n_nodes = rowptr.shape[0] - 1
    n_dest = n_nodes // N_CORES

    deg_in = np.diff(rowptr).astype(np.float64)
    deg_out = np.diff(colptr).astype(np.float64)
    with np.errstate(divide="ignore"):
        in_norm = 1.0 / np.sqrt(deg_in)
        out_norm = 1.0 / np.sqrt(deg_out)
    n_used = min(colind.shape[0], int(rowptr[-1]))
    uniform = bool(np.all(deg_in == deg_in[0]) and np.all(deg_out == deg_out[0])
                   and np.isfinite(in_norm[0]) and np.isfinite(out_norm[0]))
    if uniform:
        v_edge = None
        w_eff = (weight * np.float32(in_norm[0] * out_norm[0])).astype(np.float32)
    else:
        row_of_edge = (np.searchsorted(rowptr, np.arange(n_used),
                                       side="right") - 1)
        v_edge = (out_norm[colind[:n_used]] * in_norm[row_of_edge]).astype(
            np.float32)
        w_eff = weight.astype(np.float32)

    n_pad_rows = ((n_nodes + 3) // 4) * 4
    n_table_rows = n_pad_rows // 4
    assert n_table_rows <= 32768, "int16 gather reach exceeded"

    bias_b = np.ascontiguousarray(np.tile(bias[None, :], (128, 1)).astype(np.float32))
    iota = np.ascontiguousarray(
        np.tile(np.arange(128, dtype=np.float32)[None, :], (128, 1)))

    cores = []
    for c in range(N_CORES):
        windows = _prep_core(c, n_dest, rowptr, colind[:n_used], v_edge)
        nc, idx_all, did_all, v_all = _build_core(
            n_dest, n_table_rows, windows, uniform)
        fn, in_names, out_names, zero_outs = _make_single_runner(nc)
        ddt = _np_half() if FP16 else np.float32
        in_map = {
            "idx": _wrap_idx(idx_all),
            "did": np.ascontiguousarray(did_all.reshape(-1, 128).T.astype(ddt)),
            "wt": w_eff,
            "biasb": bias_b,
        }
        if v_all is not None:
            in_map["v"] = np.ascontiguousarray(
                v_all.reshape(-1, 128).T.astype(ddt))
        cores.append((fn, in_names, out_names, zero_outs, in_map))
    return cores, n_pad_rows, deg_in


def get_runners(x, weight, bias, rowptr, colind, colptr):
    key = (x.shape, hash(rowptr.tobytes()), hash(colind.tobytes()),
           hash(colptr.tobytes()))
    if key not in _CACHE:
        _CACHE[key] = _build_all(x, weight, bias, rowptr, colind, colptr)
    return _CACHE[key]


def run_on_device(cores, x_view, bias, deg_in):
    futs = []
    for c, (fn, in_names, out_names, zero_outs, in_map) in enumerate(cores):
        dev = jax.devices()[c]
        full = dict(in_map, x=x_view)
        dev_in = [jax.device_put(np.asarray(full[n]), dev) for n in in_names]
        dev_zero = [jax.device_put(z, dev) for z in zero_outs]
        futs.append((fn(*dev_in, *dev_zero), out_names))
    results = []
    for (out_arrs, out_names) in futs:
        jax.block_until_ready(out_arrs)
        results.append(np.asarray(out_arrs[out_names.index("out")]))
    out = np.concatenate(results, axis=0)
    zero_deg = deg_in == 0
    if zero_deg.any():
        out[zero_deg] = (np.float32(0) * np.float32(np.inf)) + bias[None, :]
    return out


def kernel(x, weight, bias, rowptr, colind, colptr, rowind):
    x = np.ascontiguousarray(np.asarray(x, np.float32))
    weight = np.asarray(weight, np.float32)
    bias = np.asarray(bias, np.float32)
    rowptr = np.asarray(rowptr, np.int64)
    colind = np.asarray(colind, np.int64)
    colptr = np.asarray(colptr, np.int64)

    n_nodes = rowptr.shape[0] - 1
    cores, n_pad_rows, deg_in = get_runners(x, weight, bias, rowptr, colind,
                                            colptr)
    dt = _np_half() if TAB16 else np.float32
    if n_pad_rows == n_nodes:
        x_view = np.ascontiguousarray(x.astype(dt)).reshape(n_nodes // 4, 4 * C)
    else:
        xp = np.zeros((n_pad_rows, C), dt)
        xp[:n_nodes] = x
        x_view = xp.reshape(n_pad_rows // 4, 4 * C)
    return run_on_device(cores, x_view, bias, deg_in)

